# revision 15
# baseline (speedup 1.0000x reference)
"""DETR-style loss kernel for Trainium2 (8 NeuronCores, data-parallel over batch).

Pipeline (mirrors the reference, which also does Hungarian matching host-side):
  1. Host: float64 cost matrices + Jonker-Volgenant LAP per batch element
     (exact replica of the reference numerics), producing target-class /
     matched-index tensors.
  2. Device (8 cores, 16 batch elements each): the heavy tensor math --
     log-sum-exp of the pen-state logits (weighted CE numerator) and matched
     L1 / SmoothL1 sums -- as fp32 elementwise + reduction work, emitting
     per-partition partial sums.
  3. Host: combine the 8x[128,5] partials in float64 into the scalar loss.

Device math notes:
  * CE numerator: sum(w * (lse - x_sel)) = sum(w * lse) - sum(w * x_sel).
    The second term is host-side. For the first, slots are pre-sorted by
    class weight into two column regions (A: w=1, B: w=0.1), so two
    Ln-activations with accum_out produce per-partition sums of ln(s)
    per region directly on the Scalar engine -- no per-slot weight tensor.
    Padded slots carry logits (0,0,0); their ln(3) contribution is
    subtracted on the host.
  * l1 + smooth_l1 per coord element a=|d|:
      max(a, 2a-beta) + (sqrt(5)*min(a, beta))^2   (beta=0.1)
"""

import numpy as np

COORD_W, WIDTH_W, CLASS_W, P0_W = 5.0, 2.0, 1.0, 2.0
BETA = 0.1
PEN_W64 = np.array([0.1, 1.0, 1.0], np.float64)
PEN_W32 = np.array([0.1, 1.0, 1.0], np.float32)

B, S, G = 128, 300, 100
N_CORES = 8
BC = B // N_CORES          # 16 batch elements per core
NSLOT = BC * S             # 4800 slots per core
# slot regions (columns per partition): A holds w=1 slots, B holds w=0.1
CA = 13                    # 128*13 = 1664 >= worst-case matched slots (1600)
CB = 38                    # 128*38 = 4864 >= 4800
NSS = CA + CB              # 51 slot columns
NM = 13                    # matched rows per partition: BC*G = 1600 -> 1664
NMAT = BC * G              # 1600
NMAT_PAD = 128 * NM        # 1664

# packed per-core input [128, NCOL] column layout
C_XL0, C_XL1 = 0, NSS * 3              # 0..153   logits (slot-major, class-minor)
C_MP0, C_MP1 = C_XL1, C_XL1 + NM * 10  # 153..283 matched preds (coords104|widths26)
C_MG0, C_MG1 = C_MP1, C_MP1 + NM * 10  # 283..413 matched gts
NCOL = C_MG1                           # 413

_SQRT5 = float(np.sqrt(np.float64(5.0)))
_LN3 = float(np.log(np.float64(3.0)))


# ---------------------------------------------------------------------------
# Host-side matching (detached in the reference too)
# ---------------------------------------------------------------------------

def _lsa_py(C):
    """Jonker-Volgenant shortest-augmenting-path LAP (rectangular, min)."""
    C = np.asarray(C, np.float64)
    transposed = C.shape[1] < C.shape[0]
    if transposed:
        C = C.T
    n, m = C.shape
    u = np.zeros(n)
    v = np.zeros(m)
    col4row = np.full(n, -1, np.int64)
    row4col = np.full(m, -1, np.int64)
    for cur in range(n):
        shortest = np.full(m, np.inf)
        path = np.full(m, -1, np.int64)
        SC = np.zeros(m, bool)
        SR = []
        minVal = 0.0
        i = cur
        sink = -1
        while sink == -1:
            SR.append(i)
            d = minVal + C[i] - u[i] - v
            upd = (~SC) & (d < shortest)
            shortest[upd] = d[upd]
            path[upd] = i
            masked = np.where(SC, np.inf, shortest)
            j = int(np.argmin(masked))
            minVal = float(masked[j])
            SC[j] = True
            if row4col[j] < 0:
                sink = j
            else:
                i = int(row4col[j])
        u[cur] += minVal
        for r in SR:
            if r != cur:
                u[r] += minVal - shortest[col4row[r]]
        v[SC] -= minVal - shortest[SC]
        j = sink
        while True:
            i = int(path[j])
            row4col[j] = i
            nxt = int(col4row[i])
            col4row[i] = j
            if i == cur:
                break
            j = nxt
    if transposed:
        return col4row.copy(), np.arange(n)
    return np.arange(n), col4row.copy()


try:
    from scipy.optimize import linear_sum_assignment as _lsa
except Exception:  # pragma: no cover
    _lsa = _lsa_py


def _softmax_f32(x):
    """fp32 softmax matching jax.nn.softmax on CPU (used by the reference)."""
    x = np.asarray(x, np.float32)
    try:
        import jax

        cpus = jax.devices("cpu")
        with jax.default_device(cpus[0]):
            import jax.numpy as jnp

            return np.asarray(jax.nn.softmax(jnp.asarray(x), axis=-1), np.float32)
    except Exception:
        m = x.max(axis=-1, keepdims=True)
        e = np.exp(x - m)
        return e / e.sum(axis=-1, keepdims=True)


def _match(strokes, pen_state_logits, targets_params, targets_labels):
    probs = _softmax_f32(pen_state_logits).astype(np.float64)
    ps = np.asarray(strokes, np.float64)
    tp = np.asarray(targets_params, np.float64)
    tl = np.asarray(targets_labels)
    tc = np.zeros((B, S), np.int32)
    pidx = np.zeros((B, G), np.int32)
    gidx = np.zeros((B, G), np.int32)
    mask = np.zeros((B, G), np.float32)
    for b in range(B):
        valid = np.where(tl[b] > 0)[0]
        nv = valid.size
        if nv == 0:
            continue
        vt = tp[b, valid]
        gt_cls = tl[b, valid]
        cost_class = -probs[b][:, gt_cls]
        cost_coord = np.abs(ps[b][:, None, :8] - vt[None, :, :8]).sum(-1)
        cost_width = np.abs(ps[b][:, None, 8:10] - vt[None, :, 8:10]).sum(-1)
        cost_p0 = np.abs(ps[b][:, None, :2] - vt[None, :, :2]).sum(-1)
        C = (
            CLASS_W * cost_class
            + COORD_W * cost_coord
            + WIDTH_W * cost_width
            + P0_W * cost_p0
        )
        r, c = _lsa(C)
        r = np.asarray(r)
        c = np.asarray(c)
        og = valid[c]
        tc[b, r] = tl[b, og]
        k = r.size
        pidx[b, :k] = r
        gidx[b, :k] = og
        mask[b, :k] = 1.0
    return tc, pidx, gidx, mask


# ---------------------------------------------------------------------------
# Per-core input packing
# ---------------------------------------------------------------------------

def _pack_inputs(strokes, pen_state_logits, targets_params, tc, pidx, gidx, mask):
    """Build per-core packed [128, NCOL] inputs plus host-side CE terms.

    Returns (in_maps, aux) with aux = dict(sum_wx, pad_corr):
      sum_wx   = sum over slots of w * x[class]              (float64)
      pad_corr = sum over cores of (padA + 0.1*padB) * ln(3) (float64)
    """
    strokes = np.asarray(strokes, np.float32)
    logits = np.asarray(pen_state_logits, np.float32)
    tp = np.asarray(targets_params, np.float32)
    in_maps = []
    sum_wx = 0.0
    pad_corr = 0.0
    for c in range(N_CORES):
        b0, b1 = c * BC, (c + 1) * BC
        lg = logits[b0:b1].reshape(NSLOT, 3)
        tcs = tc[b0:b1].reshape(NSLOT)
        w32 = PEN_W32[tcs]
        xsel = lg[np.arange(NSLOT), tcs]
        sum_wx += (w32.astype(np.float64) * xsel.astype(np.float64)).sum()

        pack = np.zeros((128, NCOL), np.float32)
        # region-sorted logits: region A (w=1) cols 0..CA-1, B cols CA..NSS-1
        xl3 = np.zeros((128, NSS, 3), np.float32)
        w1 = np.where(tcs != 0)[0]
        w0 = np.where(tcs == 0)[0]
        na, nb = w1.size, w0.size
        assert na <= 128 * CA and nb <= 128 * CB
        j = np.arange(na)
        xl3[j // CA, j % CA] = lg[w1]
        k = np.arange(nb)
        xl3[k // CB, CA + k % CB] = lg[w0]
        pack[:, C_XL0:C_XL1] = xl3.reshape(128, NSS * 3)
        pad_corr += ((128 * CA - na) + 0.1 * (128 * CB - nb)) * _LN3

        msk = mask[b0:b1][..., None]
        mp = np.take_along_axis(strokes[b0:b1], pidx[b0:b1][..., None], axis=1) * msk
        mg = np.take_along_axis(tp[b0:b1], gidx[b0:b1][..., None], axis=1) * msk
        for arr, c0 in ((mp, C_MP0), (mg, C_MG0)):
            row = np.zeros((NMAT_PAD, 10), np.float32)
            row[:NMAT] = arr.reshape(NMAT, 10)
            r3 = row.reshape(128, NM, 10)
            pack[:, c0 : c0 + NM * 8] = np.ascontiguousarray(r3[:, :, :8]).reshape(
                128, NM * 8
            )
            pack[:, c0 + NM * 8 : c0 + NM * 10] = np.ascontiguousarray(
                r3[:, :, 8:]
            ).reshape(128, NM * 2)
        in_maps.append({"pack": pack})
    return in_maps, {"sum_wx": sum_wx, "pad_corr": pad_corr}


# ---------------------------------------------------------------------------
# Device module (built once per process)
# ---------------------------------------------------------------------------

_NC = None


def _build_module_raw():
    """Raw Bass build: manual engine programs + semaphores.

    Layout: 2 input DMAs (logits on Sync, matched rows on GpSimd), all
    reductions fused into accum_out ops, Exp/Ln tables prefetched with dummy
    activations while the DMAs are in flight.
    """
    import concourse.bass as bass
    from concourse import mybir

    F32 = mybir.dt.float32
    AX = mybir.AxisListType
    OP = mybir.AluOpType
    AF = mybir.ActivationFunctionType

    nc = bass.Bass(
        "TRN2",
        target_bir_lowering=False,
        debug=False,
        enable_asserts=False,
        num_devices=N_CORES,
    )
    d_pack = nc.dram_tensor("pack", [128, NCOL], F32, kind="ExternalInput").ap()
    d_out = nc.dram_tensor("out", [128, 6], F32, kind="ExternalOutput").ap()

    P = nc.alloc_sbuf_tensor("P", [128, NCOL], F32).ap()
    sb = {}
    for n, c in [
        ("e", NSS * 3),
        ("s", NSS),
        ("d", NM * 10),
        ("a", NM * 10),
        ("m", NM * 8),
        ("q", NM * 8),
        ("parts", 6),
        ("dum0", 1),
        ("dum1", 1),
    ]:
        sb[n] = nc.alloc_sbuf_tensor(f"s_{n}", [128, c], F32).ap()

    sm = {
        n: nc.alloc_semaphore(f"sm_{n}")
        for n in ["xl", "rest", "exp", "s", "done", "vs", "out"]
    }

    cone = nc.const_aps.tensor(1.0, (128, 1), F32)

    with nc.Block() as block:

        @block.sync
        def _(sy):
            sy.dma_start(P[:, C_XL0:C_XL1], d_pack[:, C_XL0:C_XL1]).then_inc(
                sm["xl"], 16
            )
            sy.wait_ge(sm["done"], 2)
            sy.dma_start(d_out[:], sb["parts"][:]).then_inc(sm["out"], 16)

        @block.gpsimd
        def _(gp):
            gp.dma_start(P[:, C_XL1:NCOL], d_pack[:, C_XL1:NCOL]).then_inc(
                sm["rest"], 16
            )

        @block.scalar
        def _(sc):
            # Prefetch both activation tables behind the DMA transfers.
            sc.activation(sb["dum0"][:], cone, AF.Exp)
            sc.activation(sb["dum1"][:], cone, AF.Ln)
            sc.wait_ge(sm["xl"], 16)
            sc.activation(sb["e"][:], P[:, C_XL0:C_XL1], AF.Exp).then_inc(
                sm["exp"], 1
            )
            sc.wait_ge(sm["s"], 1)
            # per-partition sums of ln(s) per weight region; elementwise
            # outputs land in dead `e` slices (only accum_out is consumed)
            sc.activation(
                sb["e"][:, 0:CA], sb["s"][:, 0:CA], AF.Ln,
                accum_out=sb["parts"][:, 0:1],
            )
            sc.activation(
                sb["e"][:, CA:NSS], sb["s"][:, CA:NSS], AF.Ln,
                accum_out=sb["parts"][:, 1:2],
            ).then_inc(sm["done"], 1)

        @block.vector
        def _(ve):
            # coord l1+sl1 per element: f(a) = 2a - m + 5m^2, m = min(a, beta)
            # sum f = 2*sum(a_all) - 2*sum(a_w) - sum(m) + 5*sum(m^2),
            # combined on the host from the four accumulated partials.
            vs = sm["vs"]
            ve.wait_ge(sm["exp"], 1)
            ve.reduce_sum(
                sb["s"][:], sb["e"][:].rearrange("p (n c) -> p n c", c=3), axis=AX.X
            ).then_inc(sm["s"], 1)
            ve.wait_ge(sm["rest"], 16)
            # 1: d = mp - mg over [coords|widths] of both halves at once
            ve.tensor_sub(
                sb["d"][:], P[:, C_MP0:C_MP1], P[:, C_MG0:C_MG1]
            ).then_inc(vs)
            # 2: a = |d|, accum sum(a) over all 130 -> parts[2]
            ve.wait_ge(vs, 1)
            ve.scalar_tensor_tensor(
                sb["a"][:], sb["d"][:], -1.0, sb["d"][:], OP.mult, OP.max,
                accum_out=sb["parts"][:, 2:3],
            ).then_inc(vs)
            ac = sb["a"][:, 0 : NM * 8]
            aw = sb["a"][:, NM * 8 : NM * 10]
            # 3: m = min(a, beta) over coords, accum sum(m) -> parts[3]
            ve.wait_ge(vs, 2)
            ve.tensor_scalar(
                sb["m"][:], ac, BETA, None, OP.min, OP.add,
                accum_out=sb["parts"][:, 3:4],
            ).then_inc(vs)
            # 4: q = m*m, accum sum(m^2) -> parts[4]
            ve.wait_ge(vs, 3)
            ve.scalar_tensor_tensor(
                sb["q"][:], sb["m"][:], 0.0, sb["m"][:], OP.bypass, OP.mult,
                accum_out=sb["parts"][:, 4:5],
            )
            # 5: width sum -> parts[5]
            ve.tensor_reduce(
                sb["parts"][:, 5:6], aw, axis=AX.X, op=OP.add
            ).then_inc(sm["done"], 1)

    return nc


def _get_module():
    global _NC
    if _NC is None:
        _NC = _build_module_raw()
    return _NC


def _run_device(in_maps, trace=False):
    from concourse.bass_utils import run_bass_kernel_spmd

    nc = _get_module()
    res = run_bass_kernel_spmd(
        nc, in_maps, core_ids=list(range(N_CORES)), trace=trace
    )
    return res


# ---------------------------------------------------------------------------
# Final combine
# ---------------------------------------------------------------------------

def _combine(partials, aux, tc, mask):
    """Partial columns: 0=lnA, 1=lnB, 2=sum(a_all), 3=sum(m), 4=sum(m^2),
    5=sum(a_widths); coord sum f(a) = 2a - m + 5m^2 over coords."""
    lnA = lnB = sa = smn = sq = sw = 0.0
    for p in partials:
        p64 = np.asarray(p, np.float64)
        lnA += p64[:, 0].sum()
        lnB += p64[:, 1].sum()
        sa += p64[:, 2].sum()
        smn += p64[:, 3].sum()
        sq += p64[:, 4].sum()
        sw += p64[:, 5].sum()
    ce_wlse = lnA + 0.1 * lnB - aux["pad_corr"]
    ce_num = ce_wlse - aux["sum_wx"]
    ce_den = PEN_W64[tc].sum()
    denom = max(float(np.asarray(mask, np.float64).sum()), 1.0)
    coord_num = 2.0 * (sa - sw) - smn + 5.0 * sq
    loss = (
        CLASS_W * (ce_num / ce_den)
        + COORD_W * (coord_num / denom)
        + WIDTH_W * (sw / denom)
    )
    return np.float32(loss)


def _device_emulate(in_maps):
    """Numpy emulation of the device program (debugging aid)."""
    outs = []
    for im in in_maps:
        pk = im["pack"].astype(np.float64)
        xl = pk[:, C_XL0:C_XL1].reshape(128, NSS, 3)
        lse = np.log(np.exp(xl).sum(-1))
        lnA = lse[:, :CA].sum(-1)
        lnB = lse[:, CA:].sum(-1)
        dd = np.abs(pk[:, C_MP0:C_MP1] - pk[:, C_MG0:C_MG1])
        dc, dwv = dd[:, : NM * 8], dd[:, NM * 8 :]
        sa = dd.sum(-1)
        mm = np.minimum(dc, BETA)
        smn = mm.sum(-1)
        sq = (mm * mm).sum(-1)
        sw = dwv.sum(-1)
        outs.append(
            np.stack([lnA, lnB, sa, smn, sq, sw], axis=1).astype(np.float32)
        )
    return outs


def kernel(**inputs) -> np.ndarray:
    strokes = np.asarray(inputs["strokes"], np.float32)
    logits = np.asarray(inputs["pen_state_logits"], np.float32)
    tp = np.asarray(inputs["targets_params"], np.float32)
    tl = np.asarray(inputs["targets_labels"])

    tc, pidx, gidx, mask = _match(strokes, logits, tp, tl)
    in_maps, aux = _pack_inputs(strokes, logits, tp, tc, pidx, gidx, mask)
    res = _run_device(in_maps)
    partials = [res.results[c]["out"] for c in range(N_CORES)]
    return _combine(partials, aux, tc, mask)


# revision 17
# speedup vs baseline: 1.0923x; 1.0923x over previous
"""DETR-style loss kernel for Trainium2 (8 NeuronCores, data-parallel over batch).

Pipeline (mirrors the reference, which also does Hungarian matching host-side):
  1. Host: float64 cost matrices + Jonker-Volgenant LAP per batch element
     (exact replica of the reference numerics), producing target-class /
     matched-index tensors.
  2. Device (8 cores, 16 batch elements each): the heavy tensor math --
     log-sum-exp of the pen-state logits (weighted CE numerator) and matched
     L1 / SmoothL1 sums -- as fp32 elementwise + reduction work, emitting
     per-partition partial sums.
  3. Host: combine the 8x[128,5] partials in float64 into the scalar loss.

Device math notes:
  * CE numerator: sum(w * (lse - x_sel)) = sum(w * lse) - sum(w * x_sel).
    The second term is host-side. For the first, slots are pre-sorted by
    class weight into two column regions (A: w=1, B: w=0.1), so two
    Ln-activations with accum_out produce per-partition sums of ln(s)
    per region directly on the Scalar engine -- no per-slot weight tensor.
    Padded slots carry logits (0,0,0); their ln(3) contribution is
    subtracted on the host.
  * l1 + smooth_l1 per coord element a=|d|:
      max(a, 2a-beta) + (sqrt(5)*min(a, beta))^2   (beta=0.1)
"""

import numpy as np

COORD_W, WIDTH_W, CLASS_W, P0_W = 5.0, 2.0, 1.0, 2.0
BETA = 0.1
PEN_W64 = np.array([0.1, 1.0, 1.0], np.float64)
PEN_W32 = np.array([0.1, 1.0, 1.0], np.float32)

B, S, G = 128, 300, 100
N_CORES = 8
BC = B // N_CORES          # 16 batch elements per core
NSLOT = BC * S             # 4800 slots per core
# slot regions (columns per partition): A holds w=1 slots, B holds w=0.1
CA = 13                    # 128*13 = 1664 >= worst-case matched slots (1600)
CB = 38                    # 128*38 = 4864 >= 4800
NSS = CA + CB              # 51 slot columns
NM = 13                    # matched rows per partition: BC*G = 1600 -> 1664
NMAT = BC * G              # 1600
NMAT_PAD = 128 * NM        # 1664

# packed per-core input [128, NCOL] column layout
C_XL0, C_XL1 = 0, NSS * 3              # 0..153   logits (slot-major, class-minor)
C_MP0, C_MP1 = C_XL1, C_XL1 + NM * 10  # 153..283 matched preds (coords104|widths26)
C_MG0, C_MG1 = C_MP1, C_MP1 + NM * 10  # 283..413 matched gts
NCOL = C_MG1                           # 413

_SQRT5 = float(np.sqrt(np.float64(5.0)))
_LN3 = float(np.log(np.float64(3.0)))


# ---------------------------------------------------------------------------
# Host-side matching (detached in the reference too)
# ---------------------------------------------------------------------------

def _lsa_py(C):
    """Jonker-Volgenant shortest-augmenting-path LAP (rectangular, min)."""
    C = np.asarray(C, np.float64)
    transposed = C.shape[1] < C.shape[0]
    if transposed:
        C = C.T
    n, m = C.shape
    u = np.zeros(n)
    v = np.zeros(m)
    col4row = np.full(n, -1, np.int64)
    row4col = np.full(m, -1, np.int64)
    for cur in range(n):
        shortest = np.full(m, np.inf)
        path = np.full(m, -1, np.int64)
        SC = np.zeros(m, bool)
        SR = []
        minVal = 0.0
        i = cur
        sink = -1
        while sink == -1:
            SR.append(i)
            d = minVal + C[i] - u[i] - v
            upd = (~SC) & (d < shortest)
            shortest[upd] = d[upd]
            path[upd] = i
            masked = np.where(SC, np.inf, shortest)
            j = int(np.argmin(masked))
            minVal = float(masked[j])
            SC[j] = True
            if row4col[j] < 0:
                sink = j
            else:
                i = int(row4col[j])
        u[cur] += minVal
        for r in SR:
            if r != cur:
                u[r] += minVal - shortest[col4row[r]]
        v[SC] -= minVal - shortest[SC]
        j = sink
        while True:
            i = int(path[j])
            row4col[j] = i
            nxt = int(col4row[i])
            col4row[i] = j
            if i == cur:
                break
            j = nxt
    if transposed:
        return col4row.copy(), np.arange(n)
    return np.arange(n), col4row.copy()


try:
    from scipy.optimize import linear_sum_assignment as _lsa
except Exception:  # pragma: no cover
    _lsa = _lsa_py


def _softmax_f32(x):
    """fp32 softmax matching jax.nn.softmax on CPU (used by the reference)."""
    x = np.asarray(x, np.float32)
    try:
        import jax

        cpus = jax.devices("cpu")
        with jax.default_device(cpus[0]):
            import jax.numpy as jnp

            return np.asarray(jax.nn.softmax(jnp.asarray(x), axis=-1), np.float32)
    except Exception:
        m = x.max(axis=-1, keepdims=True)
        e = np.exp(x - m)
        return e / e.sum(axis=-1, keepdims=True)


def _match(strokes, pen_state_logits, targets_params, targets_labels):
    probs = _softmax_f32(pen_state_logits).astype(np.float64)
    ps = np.asarray(strokes, np.float64)
    tp = np.asarray(targets_params, np.float64)
    tl = np.asarray(targets_labels)
    tc = np.zeros((B, S), np.int32)
    pidx = np.zeros((B, G), np.int32)
    gidx = np.zeros((B, G), np.int32)
    mask = np.zeros((B, G), np.float32)
    for b in range(B):
        valid = np.where(tl[b] > 0)[0]
        nv = valid.size
        if nv == 0:
            continue
        vt = tp[b, valid]
        gt_cls = tl[b, valid]
        cost_class = -probs[b][:, gt_cls]
        cost_coord = np.abs(ps[b][:, None, :8] - vt[None, :, :8]).sum(-1)
        cost_width = np.abs(ps[b][:, None, 8:10] - vt[None, :, 8:10]).sum(-1)
        cost_p0 = np.abs(ps[b][:, None, :2] - vt[None, :, :2]).sum(-1)
        C = (
            CLASS_W * cost_class
            + COORD_W * cost_coord
            + WIDTH_W * cost_width
            + P0_W * cost_p0
        )
        r, c = _lsa(C)
        r = np.asarray(r)
        c = np.asarray(c)
        og = valid[c]
        tc[b, r] = tl[b, og]
        k = r.size
        pidx[b, :k] = r
        gidx[b, :k] = og
        mask[b, :k] = 1.0
    return tc, pidx, gidx, mask


# ---------------------------------------------------------------------------
# Per-core input packing
# ---------------------------------------------------------------------------

def _pack_inputs(strokes, pen_state_logits, targets_params, tc, pidx, gidx, mask):
    """Build per-core packed [128, NCOL] inputs plus host-side CE terms.

    Returns (in_maps, aux) with aux = dict(sum_wx, pad_corr):
      sum_wx   = sum over slots of w * x[class]              (float64)
      pad_corr = sum over cores of (padA + 0.1*padB) * ln(3) (float64)
    """
    strokes = np.asarray(strokes, np.float32)
    logits = np.asarray(pen_state_logits, np.float32)
    tp = np.asarray(targets_params, np.float32)
    in_maps = []
    sum_wx = 0.0
    pad_corr = 0.0
    for c in range(N_CORES):
        b0, b1 = c * BC, (c + 1) * BC
        lg = logits[b0:b1].reshape(NSLOT, 3)
        tcs = tc[b0:b1].reshape(NSLOT)
        w32 = PEN_W32[tcs]
        xsel = lg[np.arange(NSLOT), tcs]
        sum_wx += (w32.astype(np.float64) * xsel.astype(np.float64)).sum()

        pack = np.zeros((128, NCOL), np.float32)
        # region-sorted logits: region A (w=1) cols 0..CA-1, B cols CA..NSS-1
        xl3 = np.zeros((128, NSS, 3), np.float32)
        w1 = np.where(tcs != 0)[0]
        w0 = np.where(tcs == 0)[0]
        na, nb = w1.size, w0.size
        assert na <= 128 * CA and nb <= 128 * CB
        j = np.arange(na)
        xl3[j // CA, j % CA] = lg[w1]
        k = np.arange(nb)
        xl3[k // CB, CA + k % CB] = lg[w0]
        pack[:, C_XL0:C_XL1] = xl3.reshape(128, NSS * 3)
        pad_corr += ((128 * CA - na) + 0.1 * (128 * CB - nb)) * _LN3

        msk = mask[b0:b1][..., None]
        mp = np.take_along_axis(strokes[b0:b1], pidx[b0:b1][..., None], axis=1) * msk
        mg = np.take_along_axis(tp[b0:b1], gidx[b0:b1][..., None], axis=1) * msk
        for arr, c0 in ((mp, C_MP0), (mg, C_MG0)):
            row = np.zeros((NMAT_PAD, 10), np.float32)
            row[:NMAT] = arr.reshape(NMAT, 10)
            r3 = row.reshape(128, NM, 10)
            pack[:, c0 : c0 + NM * 8] = np.ascontiguousarray(r3[:, :, :8]).reshape(
                128, NM * 8
            )
            pack[:, c0 + NM * 8 : c0 + NM * 10] = np.ascontiguousarray(
                r3[:, :, 8:]
            ).reshape(128, NM * 2)
        in_maps.append({"pack": pack})
    return in_maps, {"sum_wx": sum_wx, "pad_corr": pad_corr}


# ---------------------------------------------------------------------------
# Device module (built once per process)
# ---------------------------------------------------------------------------

_NC = None


def _build_module_raw():
    """Raw Bass build: manual engine programs + semaphores.

    Layout: 2 input DMAs (logits on Sync, matched rows on GpSimd), all
    reductions fused into accum_out ops, Exp/Ln tables prefetched with dummy
    activations while the DMAs are in flight.
    """
    import concourse.bass as bass
    from concourse import mybir

    F32 = mybir.dt.float32
    AX = mybir.AxisListType
    OP = mybir.AluOpType
    AF = mybir.ActivationFunctionType

    nc = bass.Bass(
        "TRN2",
        target_bir_lowering=False,
        debug=False,
        enable_asserts=False,
        num_devices=N_CORES,
    )
    d_pack = nc.dram_tensor("pack", [128, NCOL], F32, kind="ExternalInput").ap()
    d_out = nc.dram_tensor("out", [128, 6], F32, kind="ExternalOutput").ap()

    P = nc.alloc_sbuf_tensor("P", [128, NCOL], F32).ap()
    sb = {}
    for n, c in [
        ("e", NSS * 3),
        ("s", NSS),
        ("d", NM * 10),
        ("a", NM * 10),
        ("m", NM * 8),
        ("q", NM * 8),
        ("parts", 6),
        ("dum0", 1),
        ("dum1", 1),
    ]:
        sb[n] = nc.alloc_sbuf_tensor(f"s_{n}", [128, c], F32).ap()

    sm = {
        n: nc.alloc_semaphore(f"sm_{n}")
        for n in ["xl", "mp", "mg", "exp", "s", "done", "vs", "out"]
    }

    cone = nc.const_aps.tensor(1.0, (128, 1), F32)

    with nc.Block() as block:

        @block.sync
        def _(sy):
            sy.dma_start(P[:, C_XL0:C_XL1], d_pack[:, C_XL0:C_XL1]).then_inc(
                sm["xl"], 16
            )
            sy.dma_start(P[:, C_MG0:C_MG1], d_pack[:, C_MG0:C_MG1]).then_inc(
                sm["mg"], 16
            )
            sy.wait_ge(sm["done"], 2)
            sy.dma_start(d_out[:], sb["parts"][:]).then_inc(sm["out"], 16)

        @block.scalar
        def _(sc):
            sc.dma_start(P[:, C_MP0:C_MP1], d_pack[:, C_MP0:C_MP1]).then_inc(
                sm["mp"], 16
            )
            # Prefetch both activation tables behind the DMA transfers.
            sc.activation(sb["dum0"][:], cone, AF.Exp)
            sc.activation(sb["dum1"][:], cone, AF.Ln)
            sc.wait_ge(sm["xl"], 16)
            sc.activation(sb["e"][:], P[:, C_XL0:C_XL1], AF.Exp).then_inc(
                sm["exp"], 1
            )
            sc.wait_ge(sm["s"], 1)
            # per-partition sums of ln(s) per weight region; elementwise
            # outputs land in dead `e` slices (only accum_out is consumed)
            sc.activation(
                sb["e"][:, 0:CA], sb["s"][:, 0:CA], AF.Ln,
                accum_out=sb["parts"][:, 0:1],
            )
            sc.activation(
                sb["e"][:, CA:NSS], sb["s"][:, CA:NSS], AF.Ln,
                accum_out=sb["parts"][:, 1:2],
            ).then_inc(sm["done"], 1)

        @block.vector
        def _(ve):
            # coord l1+sl1 per element: f(a) = 2a - m + 5m^2, m = min(a, beta)
            # sum f = 2*sum(a_all) - 2*sum(a_w) - sum(m) + 5*sum(m^2),
            # combined on the host from the four accumulated partials.
            vs = sm["vs"]
            ve.wait_ge(sm["exp"], 1)
            ve.reduce_sum(
                sb["s"][:], sb["e"][:].rearrange("p (n c) -> p n c", c=3), axis=AX.X
            ).then_inc(sm["s"], 1)
            ve.wait_ge(sm["mp"], 16)
            ve.wait_ge(sm["mg"], 16)
            # 1: d = mp - mg over [coords|widths] of both halves at once
            ve.tensor_sub(
                sb["d"][:], P[:, C_MP0:C_MP1], P[:, C_MG0:C_MG1]
            ).then_inc(vs)
            # 2: a = |d|, accum sum(a) over all 130 -> parts[2]
            ve.wait_ge(vs, 1)
            ve.scalar_tensor_tensor(
                sb["a"][:], sb["d"][:], -1.0, sb["d"][:], OP.mult, OP.max,
                accum_out=sb["parts"][:, 2:3],
            ).then_inc(vs)
            ac = sb["a"][:, 0 : NM * 8]
            aw = sb["a"][:, NM * 8 : NM * 10]
            # 3: m = min(a, beta) over coords, accum sum(m) -> parts[3]
            ve.wait_ge(vs, 2)
            ve.tensor_scalar(
                sb["m"][:], ac, BETA, None, OP.min, OP.add,
                accum_out=sb["parts"][:, 3:4],
            ).then_inc(vs)
            # 4: q = m*m, accum sum(m^2) -> parts[4]
            ve.wait_ge(vs, 3)
            ve.scalar_tensor_tensor(
                sb["q"][:], sb["m"][:], 0.0, sb["m"][:], OP.bypass, OP.mult,
                accum_out=sb["parts"][:, 4:5],
            )
            # 5: width sum -> parts[5]
            ve.tensor_reduce(
                sb["parts"][:, 5:6], aw, axis=AX.X, op=OP.add
            ).then_inc(sm["done"], 1)

    return nc


def _get_module():
    global _NC
    if _NC is None:
        _NC = _build_module_raw()
    return _NC


def _run_device(in_maps, trace=False):
    from concourse.bass_utils import run_bass_kernel_spmd

    nc = _get_module()
    res = run_bass_kernel_spmd(
        nc, in_maps, core_ids=list(range(N_CORES)), trace=trace
    )
    return res


# ---------------------------------------------------------------------------
# Final combine
# ---------------------------------------------------------------------------

def _combine(partials, aux, tc, mask):
    """Partial columns: 0=lnA, 1=lnB, 2=sum(a_all), 3=sum(m), 4=sum(m^2),
    5=sum(a_widths); coord sum f(a) = 2a - m + 5m^2 over coords."""
    lnA = lnB = sa = smn = sq = sw = 0.0
    for p in partials:
        p64 = np.asarray(p, np.float64)
        lnA += p64[:, 0].sum()
        lnB += p64[:, 1].sum()
        sa += p64[:, 2].sum()
        smn += p64[:, 3].sum()
        sq += p64[:, 4].sum()
        sw += p64[:, 5].sum()
    ce_wlse = lnA + 0.1 * lnB - aux["pad_corr"]
    ce_num = ce_wlse - aux["sum_wx"]
    ce_den = PEN_W64[tc].sum()
    denom = max(float(np.asarray(mask, np.float64).sum()), 1.0)
    coord_num = 2.0 * (sa - sw) - smn + 5.0 * sq
    loss = (
        CLASS_W * (ce_num / ce_den)
        + COORD_W * (coord_num / denom)
        + WIDTH_W * (sw / denom)
    )
    return np.float32(loss)


def _device_emulate(in_maps):
    """Numpy emulation of the device program (debugging aid)."""
    outs = []
    for im in in_maps:
        pk = im["pack"].astype(np.float64)
        xl = pk[:, C_XL0:C_XL1].reshape(128, NSS, 3)
        lse = np.log(np.exp(xl).sum(-1))
        lnA = lse[:, :CA].sum(-1)
        lnB = lse[:, CA:].sum(-1)
        dd = np.abs(pk[:, C_MP0:C_MP1] - pk[:, C_MG0:C_MG1])
        dc, dwv = dd[:, : NM * 8], dd[:, NM * 8 :]
        sa = dd.sum(-1)
        mm = np.minimum(dc, BETA)
        smn = mm.sum(-1)
        sq = (mm * mm).sum(-1)
        sw = dwv.sum(-1)
        outs.append(
            np.stack([lnA, lnB, sa, smn, sq, sw], axis=1).astype(np.float32)
        )
    return outs


def kernel(**inputs) -> np.ndarray:
    strokes = np.asarray(inputs["strokes"], np.float32)
    logits = np.asarray(inputs["pen_state_logits"], np.float32)
    tp = np.asarray(inputs["targets_params"], np.float32)
    tl = np.asarray(inputs["targets_labels"])

    tc, pidx, gidx, mask = _match(strokes, logits, tp, tl)
    in_maps, aux = _pack_inputs(strokes, logits, tp, tc, pidx, gidx, mask)
    res = _run_device(in_maps)
    partials = [res.results[c]["out"] for c in range(N_CORES)]
    return _combine(partials, aux, tc, mask)


# revision 18
# speedup vs baseline: 1.1169x; 1.0226x over previous
"""DETR-style loss kernel for Trainium2 (8 NeuronCores, data-parallel over batch).

Pipeline (mirrors the reference, which also does Hungarian matching host-side):
  1. Host: float64 cost matrices + Jonker-Volgenant LAP per batch element
     (exact replica of the reference numerics), producing target-class /
     matched-index tensors.
  2. Device (8 cores, 16 batch elements each): the heavy tensor math --
     log-sum-exp of the pen-state logits (weighted CE numerator) and matched
     L1 / SmoothL1 sums -- as fp32 elementwise + reduction work, emitting
     per-partition partial sums.
  3. Host: combine the 8x[128,5] partials in float64 into the scalar loss.

Device math notes:
  * CE numerator: sum(w * (lse - x_sel)) = sum(w * lse) - sum(w * x_sel).
    The second term is host-side. For the first, slots are pre-sorted by
    class weight into two column regions (A: w=1, B: w=0.1), so two
    Ln-activations with accum_out produce per-partition sums of ln(s)
    per region directly on the Scalar engine -- no per-slot weight tensor.
    Padded slots carry logits (0,0,0); their ln(3) contribution is
    subtracted on the host.
  * l1 + smooth_l1 per coord element a=|d|:
      max(a, 2a-beta) + (sqrt(5)*min(a, beta))^2   (beta=0.1)
"""

import numpy as np

COORD_W, WIDTH_W, CLASS_W, P0_W = 5.0, 2.0, 1.0, 2.0
BETA = 0.1
PEN_W64 = np.array([0.1, 1.0, 1.0], np.float64)
PEN_W32 = np.array([0.1, 1.0, 1.0], np.float32)

B, S, G = 128, 300, 100
N_CORES = 8
BC = B // N_CORES          # 16 batch elements per core
NSLOT = BC * S             # 4800 slots per core
# slot regions (columns per partition): A holds w=1 slots, B holds w=0.1
CA = 13                    # 128*13 = 1664 >= worst-case matched slots (1600)
CB = 38                    # 128*38 = 4864 >= 4800
NSS = CA + CB              # 51 slot columns
NM = 13                    # matched rows per partition: BC*G = 1600 -> 1664
NMAT = BC * G              # 1600
NMAT_PAD = 128 * NM        # 1664

# packed per-core input [128, NCOL] column layout
C_XL0, C_XL1 = 0, NSS * 3              # 0..153   logits (slot-major, class-minor)
C_MP0, C_MP1 = C_XL1, C_XL1 + NM * 10  # 153..283 matched preds (coords104|widths26)
C_MG0, C_MG1 = C_MP1, C_MP1 + NM * 10  # 283..413 matched gts
NCOL = C_MG1                           # 413

_SQRT5 = float(np.sqrt(np.float64(5.0)))
_LN3 = float(np.log(np.float64(3.0)))


# ---------------------------------------------------------------------------
# Host-side matching (detached in the reference too)
# ---------------------------------------------------------------------------

def _lsa_py(C):
    """Jonker-Volgenant shortest-augmenting-path LAP (rectangular, min)."""
    C = np.asarray(C, np.float64)
    transposed = C.shape[1] < C.shape[0]
    if transposed:
        C = C.T
    n, m = C.shape
    u = np.zeros(n)
    v = np.zeros(m)
    col4row = np.full(n, -1, np.int64)
    row4col = np.full(m, -1, np.int64)
    for cur in range(n):
        shortest = np.full(m, np.inf)
        path = np.full(m, -1, np.int64)
        SC = np.zeros(m, bool)
        SR = []
        minVal = 0.0
        i = cur
        sink = -1
        while sink == -1:
            SR.append(i)
            d = minVal + C[i] - u[i] - v
            upd = (~SC) & (d < shortest)
            shortest[upd] = d[upd]
            path[upd] = i
            masked = np.where(SC, np.inf, shortest)
            j = int(np.argmin(masked))
            minVal = float(masked[j])
            SC[j] = True
            if row4col[j] < 0:
                sink = j
            else:
                i = int(row4col[j])
        u[cur] += minVal
        for r in SR:
            if r != cur:
                u[r] += minVal - shortest[col4row[r]]
        v[SC] -= minVal - shortest[SC]
        j = sink
        while True:
            i = int(path[j])
            row4col[j] = i
            nxt = int(col4row[i])
            col4row[i] = j
            if i == cur:
                break
            j = nxt
    if transposed:
        return col4row.copy(), np.arange(n)
    return np.arange(n), col4row.copy()


try:
    from scipy.optimize import linear_sum_assignment as _lsa
except Exception:  # pragma: no cover
    _lsa = _lsa_py


def _softmax_f32(x):
    """fp32 softmax matching jax.nn.softmax on CPU (used by the reference)."""
    x = np.asarray(x, np.float32)
    try:
        import jax

        cpus = jax.devices("cpu")
        with jax.default_device(cpus[0]):
            import jax.numpy as jnp

            return np.asarray(jax.nn.softmax(jnp.asarray(x), axis=-1), np.float32)
    except Exception:
        m = x.max(axis=-1, keepdims=True)
        e = np.exp(x - m)
        return e / e.sum(axis=-1, keepdims=True)


def _match(strokes, pen_state_logits, targets_params, targets_labels):
    probs = _softmax_f32(pen_state_logits).astype(np.float64)
    ps = np.asarray(strokes, np.float64)
    tp = np.asarray(targets_params, np.float64)
    tl = np.asarray(targets_labels)
    tc = np.zeros((B, S), np.int32)
    pidx = np.zeros((B, G), np.int32)
    gidx = np.zeros((B, G), np.int32)
    mask = np.zeros((B, G), np.float32)
    for b in range(B):
        valid = np.where(tl[b] > 0)[0]
        nv = valid.size
        if nv == 0:
            continue
        vt = tp[b, valid]
        gt_cls = tl[b, valid]
        cost_class = -probs[b][:, gt_cls]
        cost_coord = np.abs(ps[b][:, None, :8] - vt[None, :, :8]).sum(-1)
        cost_width = np.abs(ps[b][:, None, 8:10] - vt[None, :, 8:10]).sum(-1)
        cost_p0 = np.abs(ps[b][:, None, :2] - vt[None, :, :2]).sum(-1)
        C = (
            CLASS_W * cost_class
            + COORD_W * cost_coord
            + WIDTH_W * cost_width
            + P0_W * cost_p0
        )
        r, c = _lsa(C)
        r = np.asarray(r)
        c = np.asarray(c)
        og = valid[c]
        tc[b, r] = tl[b, og]
        k = r.size
        pidx[b, :k] = r
        gidx[b, :k] = og
        mask[b, :k] = 1.0
    return tc, pidx, gidx, mask


# ---------------------------------------------------------------------------
# Per-core input packing
# ---------------------------------------------------------------------------

def _pack_inputs(strokes, pen_state_logits, targets_params, tc, pidx, gidx, mask):
    """Build per-core packed [128, NCOL] inputs plus host-side CE terms.

    Returns (in_maps, aux) with aux = dict(sum_wx, pad_corr):
      sum_wx   = sum over slots of w * x[class]              (float64)
      pad_corr = sum over cores of (padA + 0.1*padB) * ln(3) (float64)
    """
    strokes = np.asarray(strokes, np.float32)
    logits = np.asarray(pen_state_logits, np.float32)
    tp = np.asarray(targets_params, np.float32)
    in_maps = []
    sum_wx = 0.0
    pad_corr = 0.0
    for c in range(N_CORES):
        b0, b1 = c * BC, (c + 1) * BC
        lg = logits[b0:b1].reshape(NSLOT, 3)
        tcs = tc[b0:b1].reshape(NSLOT)
        w32 = PEN_W32[tcs]
        xsel = lg[np.arange(NSLOT), tcs]
        sum_wx += (w32.astype(np.float64) * xsel.astype(np.float64)).sum()

        pack = np.zeros((128, NCOL), np.float32)
        # region-sorted logits: region A (w=1) cols 0..CA-1, B cols CA..NSS-1
        xl3 = np.zeros((128, NSS, 3), np.float32)
        w1 = np.where(tcs != 0)[0]
        w0 = np.where(tcs == 0)[0]
        na, nb = w1.size, w0.size
        assert na <= 128 * CA and nb <= 128 * CB
        j = np.arange(na)
        xl3[j // CA, j % CA] = lg[w1]
        k = np.arange(nb)
        xl3[k // CB, CA + k % CB] = lg[w0]
        pack[:, C_XL0:C_XL1] = xl3.reshape(128, NSS * 3)
        pad_corr += ((128 * CA - na) + 0.1 * (128 * CB - nb)) * _LN3

        msk = mask[b0:b1][..., None]
        mp = np.take_along_axis(strokes[b0:b1], pidx[b0:b1][..., None], axis=1) * msk
        mg = np.take_along_axis(tp[b0:b1], gidx[b0:b1][..., None], axis=1) * msk
        for arr, c0 in ((mp, C_MP0), (mg, C_MG0)):
            row = np.zeros((NMAT_PAD, 10), np.float32)
            row[:NMAT] = arr.reshape(NMAT, 10)
            r3 = row.reshape(128, NM, 10)
            pack[:, c0 : c0 + NM * 8] = np.ascontiguousarray(r3[:, :, :8]).reshape(
                128, NM * 8
            )
            pack[:, c0 + NM * 8 : c0 + NM * 10] = np.ascontiguousarray(
                r3[:, :, 8:]
            ).reshape(128, NM * 2)
        in_maps.append({"pack": pack})
    return in_maps, {"sum_wx": sum_wx, "pad_corr": pad_corr}


# ---------------------------------------------------------------------------
# Device module (built once per process)
# ---------------------------------------------------------------------------

_NC = None


def _build_module_raw():
    """Raw Bass build: manual engine programs + semaphores.

    Layout: 2 input DMAs (logits on Sync, matched rows on GpSimd), all
    reductions fused into accum_out ops, Exp/Ln tables prefetched with dummy
    activations while the DMAs are in flight.
    """
    import concourse.bass as bass
    from concourse import mybir

    F32 = mybir.dt.float32
    AX = mybir.AxisListType
    OP = mybir.AluOpType
    AF = mybir.ActivationFunctionType

    nc = bass.Bass(
        "TRN2",
        target_bir_lowering=False,
        debug=False,
        enable_asserts=False,
        num_devices=N_CORES,
    )
    d_pack = nc.dram_tensor("pack", [128, NCOL], F32, kind="ExternalInput").ap()
    d_out = nc.dram_tensor("out", [128, 6], F32, kind="ExternalOutput").ap()

    P = nc.alloc_sbuf_tensor("P", [128, NCOL], F32).ap()
    sb = {}
    for n, c in [
        ("e", NSS * 3),
        ("s", NSS),
        ("d", NM * 10),
        ("a", NM * 10),
        ("m", NM * 8),
        ("q", NM * 8),
        ("parts", 6),
        ("dum0", 1),
        ("dum1", 1),
    ]:
        sb[n] = nc.alloc_sbuf_tensor(f"s_{n}", [128, c], F32).ap()

    sm = {
        n: nc.alloc_semaphore(f"sm_{n}")
        for n in ["xl", "mpg", "exp", "s", "done", "vs", "out"]
    }

    cone = nc.const_aps.tensor(1.0, (128, 1), F32)

    with nc.Block() as block:

        @block.sync
        def _(sy):
            sy.dma_start(P[:, C_XL0:C_XL1], d_pack[:, C_XL0:C_XL1]).then_inc(
                sm["xl"], 16
            )
            sy.wait_ge(sm["done"], 2)
            sy.dma_start(d_out[:], sb["parts"][:]).then_inc(sm["out"], 16)

        @block.scalar
        def _(sc):
            sc.dma_start(P[:, C_MP0:C_MG1], d_pack[:, C_MP0:C_MG1]).then_inc(
                sm["mpg"], 16
            )
            # Prefetch both activation tables behind the DMA transfers.
            sc.activation(sb["dum0"][:], cone, AF.Exp)
            sc.activation(sb["dum1"][:], cone, AF.Ln)
            sc.wait_ge(sm["xl"], 16)
            sc.activation(sb["e"][:], P[:, C_XL0:C_XL1], AF.Exp).then_inc(
                sm["exp"], 1
            )
            sc.wait_ge(sm["s"], 1)
            # per-partition sums of ln(s) per weight region; elementwise
            # outputs land in dead `e` slices (only accum_out is consumed)
            sc.activation(
                sb["e"][:, 0:CA], sb["s"][:, 0:CA], AF.Ln,
                accum_out=sb["parts"][:, 0:1],
            )
            sc.activation(
                sb["e"][:, CA:NSS], sb["s"][:, CA:NSS], AF.Ln,
                accum_out=sb["parts"][:, 1:2],
            ).then_inc(sm["done"], 1)

        @block.vector
        def _(ve):
            # coord l1+sl1 per element: f(a) = 2a - m + 5m^2, m = min(a, beta)
            # sum f = 2*sum(a_all) - 2*sum(a_w) - sum(m) + 5*sum(m^2),
            # combined on the host from the four accumulated partials.
            vs = sm["vs"]
            ve.wait_ge(sm["mpg"], 16)
            # 1: d = mp - mg over [coords|widths] of both halves at once
            ve.tensor_sub(
                sb["d"][:], P[:, C_MP0:C_MP1], P[:, C_MG0:C_MG1]
            ).then_inc(vs)
            ve.wait_ge(sm["exp"], 1)
            ve.reduce_sum(
                sb["s"][:], sb["e"][:].rearrange("p (n c) -> p n c", c=3), axis=AX.X
            ).then_inc(sm["s"], 1)
            # 2: a = |d|, accum sum(a) over all 130 -> parts[2]
            ve.wait_ge(vs, 1)
            ve.scalar_tensor_tensor(
                sb["a"][:], sb["d"][:], -1.0, sb["d"][:], OP.mult, OP.max,
                accum_out=sb["parts"][:, 2:3],
            ).then_inc(vs)
            ac = sb["a"][:, 0 : NM * 8]
            aw = sb["a"][:, NM * 8 : NM * 10]
            # 3: m = min(a, beta) over coords, accum sum(m) -> parts[3]
            ve.wait_ge(vs, 2)
            ve.tensor_scalar(
                sb["m"][:], ac, BETA, None, OP.min, OP.add,
                accum_out=sb["parts"][:, 3:4],
            ).then_inc(vs)
            # 4: q = m*m, accum sum(m^2) -> parts[4]
            ve.wait_ge(vs, 3)
            ve.scalar_tensor_tensor(
                sb["q"][:], sb["m"][:], 0.0, sb["m"][:], OP.bypass, OP.mult,
                accum_out=sb["parts"][:, 4:5],
            )
            # 5: width sum -> parts[5]
            ve.tensor_reduce(
                sb["parts"][:, 5:6], aw, axis=AX.X, op=OP.add
            ).then_inc(sm["done"], 1)

    return nc


def _get_module():
    global _NC
    if _NC is None:
        _NC = _build_module_raw()
    return _NC


def _run_device(in_maps, trace=False):
    from concourse.bass_utils import run_bass_kernel_spmd

    nc = _get_module()
    res = run_bass_kernel_spmd(
        nc, in_maps, core_ids=list(range(N_CORES)), trace=trace
    )
    return res


# ---------------------------------------------------------------------------
# Final combine
# ---------------------------------------------------------------------------

def _combine(partials, aux, tc, mask):
    """Partial columns: 0=lnA, 1=lnB, 2=sum(a_all), 3=sum(m), 4=sum(m^2),
    5=sum(a_widths); coord sum f(a) = 2a - m + 5m^2 over coords."""
    lnA = lnB = sa = smn = sq = sw = 0.0
    for p in partials:
        p64 = np.asarray(p, np.float64)
        lnA += p64[:, 0].sum()
        lnB += p64[:, 1].sum()
        sa += p64[:, 2].sum()
        smn += p64[:, 3].sum()
        sq += p64[:, 4].sum()
        sw += p64[:, 5].sum()
    ce_wlse = lnA + 0.1 * lnB - aux["pad_corr"]
    ce_num = ce_wlse - aux["sum_wx"]
    ce_den = PEN_W64[tc].sum()
    denom = max(float(np.asarray(mask, np.float64).sum()), 1.0)
    coord_num = 2.0 * (sa - sw) - smn + 5.0 * sq
    loss = (
        CLASS_W * (ce_num / ce_den)
        + COORD_W * (coord_num / denom)
        + WIDTH_W * (sw / denom)
    )
    return np.float32(loss)


def _device_emulate(in_maps):
    """Numpy emulation of the device program (debugging aid)."""
    outs = []
    for im in in_maps:
        pk = im["pack"].astype(np.float64)
        xl = pk[:, C_XL0:C_XL1].reshape(128, NSS, 3)
        lse = np.log(np.exp(xl).sum(-1))
        lnA = lse[:, :CA].sum(-1)
        lnB = lse[:, CA:].sum(-1)
        dd = np.abs(pk[:, C_MP0:C_MP1] - pk[:, C_MG0:C_MG1])
        dc, dwv = dd[:, : NM * 8], dd[:, NM * 8 :]
        sa = dd.sum(-1)
        mm = np.minimum(dc, BETA)
        smn = mm.sum(-1)
        sq = (mm * mm).sum(-1)
        sw = dwv.sum(-1)
        outs.append(
            np.stack([lnA, lnB, sa, smn, sq, sw], axis=1).astype(np.float32)
        )
    return outs


def kernel(**inputs) -> np.ndarray:
    strokes = np.asarray(inputs["strokes"], np.float32)
    logits = np.asarray(inputs["pen_state_logits"], np.float32)
    tp = np.asarray(inputs["targets_params"], np.float32)
    tl = np.asarray(inputs["targets_labels"])

    tc, pidx, gidx, mask = _match(strokes, logits, tp, tl)
    in_maps, aux = _pack_inputs(strokes, logits, tp, tc, pidx, gidx, mask)
    res = _run_device(in_maps)
    partials = [res.results[c]["out"] for c in range(N_CORES)]
    return _combine(partials, aux, tc, mask)
